# revision 33
# baseline (speedup 1.0000x reference)
"""Trainium2 Bass kernel: 2048-point Hadamard transform (GEMM vs Sylvester H).

out = (value @ H2048) * (1/sqrt(2048)),  value: (32768, 2048) f32,
H2048 symmetric +-1 Sylvester Hadamard (== the `weight` input).

Kronecker factorization H2048 = H128 (x) H16 (n = a*16+b, m = ma*16+mb):
    out[t, ma*16+mb] = sum_a H128[a,ma] * (sum_b H16[b,mb] * V[t, a*16+b])

The host pre-transposes each core's token slice to VT [2048, 4096] bf16 so
the contraction index starts on SBUF partitions (p = n mod 128 = (a%8)*16+b,
g = n div 128 = a div 8). Per 128-token tile the device then needs only
3 PE passes (all N=128, bf16, ~71ns/op pipelined):
  1. 16x MM   lhsT=VT-block, rhs=R1p (col-permuted (I8 (x) H16)/16)
              -> m1[t, (mb, ar)] f32 PSUM, DVE-drained into Y[t, mb, a] bf16
  2. 16x tp   Y[:, mb, :] -> [a, t] bf16 PSUM, drained to SBUF (DVE/ACT)
  3. 16x MM   lhsT=[a, t], rhs=H128 * 2^-1.5 -> m2[t, ma] f32 PSUM,
              ACT-drained (contiguous) into OUT[t, (mb, ma)] bf16
The 1/sqrt(2048) scale is folded into the constants; OUT is stored in
scrambled (mb, ma) column order so every PSUM drain is contiguous -- the
host applies the inverse permutation during its bf16->f32 upcast. Total
error is the bf16 rounding of data + constants (~3.5e-3 relative).

Engine budget per core (4096 tokens, 32 tiles): PE ~100us (1536 MM/tp),
DVE ~110us (m1 drains + 3/4 tp2 drains), ACT ~105us (m2 drains + 1/4 tp2),
DMA 32MB. Matmuls must drain fp32 to PSUM on TRN2, so the two f32 drain
sets (2 x 2048 elems/tile at 1 elem/cycle) are the structural floor.

Sharding: data-parallel on the token dim across 8 cores (4096 tokens each).
walrus allows only one semaphore wait per PE-queue instruction; the bacc
legalization passes (move_matmul_waits_to_ldweights +
generate_event_semaphores) are run on the module to make the Tile-generated
sync legal.
"""

import numpy as np
import ml_dtypes

import concourse.bass as bass
import concourse.mybir as mybir
import concourse.tile as tile
from concourse.bass import ts
from concourse.bass_utils import run_bass_kernel_spmd

N_CORES = 8
T_FULL = 32768
N = 2048
T_CORE = T_FULL // N_CORES  # 4096
P = 128
ST = 512  # tokens per super-tile (one input DMA)

BF16 = mybir.dt.bfloat16
F32 = mybir.dt.float32


def _sylvester(n: int) -> np.ndarray:
    H = np.array([[1.0]], dtype=np.float64)
    while H.shape[0] < n:
        H = np.block([[H, H], [H, -H]])
    return H


def _host_consts() -> np.ndarray:
    """[128, 384] bf16: [R1p | H128s | I128].

    R1p = (I8 (x) H16)/16 with output columns permuted to j = mb*8 + ar so
    the m1 -> Y copy APs keep an 8-element contiguous innermost run.
    The total 1/sqrt(2048) output scale is folded into the constants:
    2^-4 into R1p (exact in bf16) and 2^-1.5 into H128s (~1e-4 rounding).
    """
    H16 = _sylvester(16)
    H128 = _sylvester(128)
    R1p = np.zeros((128, 128))
    for ar in range(8):
        for b in range(16):
            for mb in range(16):
                R1p[ar * 16 + b, mb * 8 + ar] = H16[b, mb] / 16.0
    ident = np.eye(128)
    return np.concatenate(
        [R1p, H128 * 2.0 ** -1.5, ident], axis=1
    ).astype(ml_dtypes.bfloat16)


def build_bass(t_core: int = T_CORE) -> bass.Bass:
    n_super = t_core // ST  # 8
    nc = bass.Bass()
    vt_p = nc.declare_dram_parameter("value_t", [N, t_core], BF16, isOutput=False)
    consts_p = nc.declare_dram_parameter("consts", [P, 3 * P], BF16, isOutput=False)
    out_p = nc.declare_dram_parameter("out", [t_core, N], BF16, isOutput=True)

    vt_view = vt_p.rearrange("(g p) t -> p g t", p=P)  # n = g*128 + p

    with tile.TileContext(nc) as tc:
        with (
            tc.tile_pool(name="consts", bufs=1) as consts,
            tc.tile_pool(name="vpool", bufs=3) as vpool,
            tc.tile_pool(name="ypool", bufs=4) as ypool,
            tc.tile_pool(name="v2pool", bufs=4) as v2pool,
            tc.tile_pool(name="opool", bufs=4) as opool,
            tc.tile_pool(name="m1psum", bufs=2, space="PSUM") as m1psum,
            tc.tile_pool(name="tpsum", bufs=2, space="PSUM") as tpsum,
            tc.tile_pool(name="m2psum", bufs=2, space="PSUM") as m2psum,
        ):
            CONSTS = consts.tile([P, 3 * P], BF16, tag="consts")
            # scalar (HWDGE) queue: runs parallel to the V loads on sync
            nc.scalar.dma_start(out=CONSTS, in_=consts_p[:, :])
            R1P = CONSTS[:, 0:P]
            H128B = CONSTS[:, P : 2 * P]
            IDENT = CONSTS[:, 2 * P : 3 * P]

            n_tiles = t_core // P

            V = None  # current super-tile [128, 16, 512]
            Ys = [None] * n_tiles  # per-tile stage-1 results [128, 16(mb), 128(a)]

            def emit_stage1(it):
                """MM1s for tile `it` -> Y[t, mb, a] in SBUF."""
                tsub = it % (ST // P)
                Y = ypool.tile([P, 16, P], BF16, tag="y")
                Yv = Y.rearrange("p m (g a) -> p g m a", g=16)  # [p,16,16,8]
                for G in range(2):
                    m1 = m1psum.tile([P, 8, P], F32, tag="m1")
                    for gg in range(8):
                        g = G * 8 + gg
                        nc.tensor.matmul(
                            m1[:, gg],
                            V[:, g, ts(tsub, P)],
                            R1P,
                            start=True,
                            stop=True,
                        )
                    # m1 free = (gg, mb, ar); Y slice free = (g, mb, ar).
                    # fp32 PSUM source -> 1x mode; strides are free at 1x.
                    # ACT is ~4x slower than DVE on strided copies, so all
                    # Y-assembly stays on DVE; FD=1024 amortizes the op
                    # overhead.
                    nc.vector.tensor_copy(
                        out=Yv[:, ts(G, 8)],
                        in_=m1.rearrange("p q (m a) -> p q m a", m=16),
                    )
                Ys[it] = Y

            def emit_stage2(it, split_store=False):
                """Transposes + MM2s for tile `it` -> store to DRAM.

                OUT free layout is (mb, ma) -- NOT final m = ma*16+mb order.
                Every PSUM drain is then contiguous; the host fixes the
                permutation during its bf16->f32 upcast pass.
                """
                Y = Ys[it]
                OUT = opool.tile([P, N], BF16, tag="o")
                for H in range(2):
                    tp2 = tpsum.tile([P, 8, P], BF16, tag="tp2")
                    for mm in range(8):
                        mb = H * 8 + mm
                        nc.tensor.transpose(tp2[:, mm], Y[:, mb, :], IDENT)
                    vt2 = v2pool.tile([P, 8, P], BF16, tag="vt2")
                    # ~1 in 4 tp2 drains go to ACT to balance DVE/ACT load
                    if H == 1 and it % 2 == 1:
                        nc.scalar.activation(
                            out=vt2,
                            in_=tp2,
                            func=mybir.ActivationFunctionType.Copy,
                        )
                    else:
                        nc.vector.tensor_copy(out=vt2, in_=tp2)
                    for q in range(2):
                        m2 = m2psum.tile([P, 4, P], F32, tag="m2")
                        for mm in range(4):
                            nc.tensor.matmul(
                                m2[:, mm],
                                vt2[:, q * 4 + mm],
                                H128B,
                                start=True,
                                stop=True,
                            )
                        Q = H * 2 + q
                        nc.scalar.activation(
                            out=OUT[:, ts(Q, 512)],
                            in_=m2,
                            func=mybir.ActivationFunctionType.Copy,
                        )
                        if split_store:
                            nc.gpsimd.dma_start(
                                out=out_p[ts(it, P), ts(Q, 512)],
                                in_=OUT[:, ts(Q, 512)],
                            )
                Ys[it] = None
                # stores ride the idle GpSimd (SWDGE) queue so their trigger
                # cost never delays the input prefetches on the Sync queue
                if not split_store:
                    nc.gpsimd.dma_start(out=out_p[ts(it, P), :], in_=OUT)

            # software pipeline: stage1(it) runs one tile ahead of stage2(it-1)
            # so PE never waits on the DVE copies that assemble Y. Input
            # super-tiles are prefetched one full super-tile (~13us of
            # compute) ahead of use.
            VS = [None] * n_super

            def issue_load(st):
                vtile = vpool.tile([P, 16, ST], BF16, tag="v")
                VS[st] = vtile
                if st == 0:
                    # split the first load into 256KB (g-half, t-quarter)
                    # chunks, in consumption order, so tile 0's first
                    # m1-group (g0-7, t0-127) starts after one chunk
                    for q in range(4):
                        for gh in range(2):
                            nc.sync.dma_start(
                                out=VS[st][:, ts(gh, 8), ts(q, P)],
                                in_=vt_view[:, ts(gh, 8), ts(q, P)],
                            )
                else:
                    nc.sync.dma_start(
                        out=VS[st], in_=vt_view[:, :, ts(st, ST)]
                    )

            issue_load(0)
            for st in range(n_super):
                V = VS[st]
                if st + 1 < n_super:
                    issue_load(st + 1)
                for tsub in range(ST // P):
                    it = st * (ST // P) + tsub
                    emit_stage1(it)
                    if it > 0:
                        emit_stage2(it - 1)
            emit_stage2(n_tiles - 1, split_store=True)

    # walrus allows at most 1 sem wait per PE-queue instruction; tile_legalize
    # pre-splits bf16 matmuls into Ldweights+Matmult, so run the bacc
    # legalization passes that spread/split the waits legally.
    import bass_rust

    bass_rust.move_matmul_waits_to_ldweights(nc.m)
    bass_rust.generate_event_semaphores(nc)
    return nc


_CACHE = {}


def _make_in_maps(inputs) -> list:
    value = np.asarray(inputs["value"])
    assert value.shape == (T_FULL, N), value.shape
    vb = value.astype(ml_dtypes.bfloat16)
    consts = _host_consts()
    in_maps = []
    for c in range(N_CORES):
        vt = np.ascontiguousarray(vb[c * T_CORE : (c + 1) * T_CORE].T)
        in_maps.append({"value_t": vt, "consts": consts})
    return in_maps


def _probe_ok(out: np.ndarray, inputs, n_rows: int = 3) -> bool:
    """Spot-check a few rows on the host against the *actual* weight input.

    Catches silent HW corruption and also the (unexpected) case of a
    non-Hadamard weight, falling back to the generic path.
    """
    value = np.asarray(inputs["value"])
    weight = np.asarray(inputs["weight"], dtype=np.float32)
    rows = np.linspace(0, T_FULL - 1, n_rows).astype(int)
    scale = np.float32(1.0 / np.sqrt(np.float32(weight.shape[0])))
    want = (np.asarray(value[rows], dtype=np.float32) @ weight) * scale
    got = out[rows]
    denom = max(float(np.abs(want).max()), 1e-30)
    rel = float(np.abs(got - want).max()) / denom
    return rel < 1.5e-2


def kernel(**inputs) -> np.ndarray:
    if "nc" not in _CACHE:
        _CACHE["nc"] = build_bass(T_CORE)
    nc = _CACHE["nc"]

    in_maps = _make_in_maps(inputs)
    try:
        out = None
        for attempt in range(2):
            res = run_bass_kernel_spmd(nc, in_maps, list(range(N_CORES)))
            o = np.concatenate([r["out"] for r in res.results], axis=0)
            # device emits (mb, ma)-ordered columns; final m = ma*16 + mb
            o = o.reshape(T_FULL, 16, 128).swapaxes(1, 2)
            o = np.ascontiguousarray(o, dtype=np.float32).reshape(T_FULL, N)
            if _probe_ok(o, inputs):
                out = o
                break
            print("kernel: probe mismatch on attempt", attempt, flush=True)
        if out is None:
            raise RuntimeError("bass kernel failed host probe twice")
        return out
    except Exception:
        import traceback

        traceback.print_exc()
        print("kernel: falling back to jax path", flush=True)
        # fallback: data-parallel GEMM via XLA on the 8 NeuronCores
        import jax
        import jax.numpy as jnp

        value = np.asarray(inputs["value"], dtype=np.float32)
        devs = jax.devices()[:N_CORES]
        scale = np.float32(1.0 / np.sqrt(np.float32(N)))
        w = np.asarray(inputs["weight"], dtype=np.float32)
        outs = []
        for c in range(N_CORES):
            d = devs[c % len(devs)]
            f = jax.jit(lambda a, b: jnp.dot(a, b) * scale, device=d)
            outs.append(f(value[c * T_CORE : (c + 1) * T_CORE], w))
        return np.concatenate([np.asarray(o) for o in outs], axis=0).astype(
            np.float32
        )


# revision 34
# speedup vs baseline: 1.0061x; 1.0061x over previous
"""Trainium2 Bass kernel: 2048-point Hadamard transform (GEMM vs Sylvester H).

out = (value @ H2048) * (1/sqrt(2048)),  value: (32768, 2048) f32,
H2048 symmetric +-1 Sylvester Hadamard (== the `weight` input).

Kronecker factorization H2048 = H128 (x) H16 (n = a*16+b, m = ma*16+mb):
    out[t, ma*16+mb] = sum_a H128[a,ma] * (sum_b H16[b,mb] * V[t, a*16+b])

The host pre-transposes each core's token slice to VT [2048, 4096] bf16 so
the contraction index starts on SBUF partitions (p = n mod 128 = (a%8)*16+b,
g = n div 128 = a div 8). Per 128-token tile the device then needs only
3 PE passes (all N=128, bf16, ~71ns/op pipelined):
  1. 16x MM   lhsT=VT-block, rhs=R1p (col-permuted (I8 (x) H16)/16)
              -> m1[t, (mb, ar)] f32 PSUM, DVE-drained into Y[t, mb, a] bf16
  2. 16x tp   Y[:, mb, :] -> [a, t] bf16 PSUM, drained to SBUF (DVE/ACT)
  3. 16x MM   lhsT=[a, t], rhs=H128 * 2^-1.5 -> m2[t, ma] f32 PSUM,
              ACT-drained (contiguous) into OUT[t, (mb, ma)] bf16
The 1/sqrt(2048) scale is folded into the constants; OUT is stored in
scrambled (mb, ma) column order so every PSUM drain is contiguous -- the
host applies the inverse permutation during its bf16->f32 upcast. Total
error is the bf16 rounding of data + constants (~3.5e-3 relative).

Engine budget per core (4096 tokens, 32 tiles): PE ~100us (1536 MM/tp),
DVE ~110us (m1 drains + 3/4 tp2 drains), ACT ~105us (m2 drains + 1/4 tp2),
DMA 32MB. Matmuls must drain fp32 to PSUM on TRN2, so the two f32 drain
sets (2 x 2048 elems/tile at 1 elem/cycle) are the structural floor.

Sharding: data-parallel on the token dim across 8 cores (4096 tokens each).
walrus allows only one semaphore wait per PE-queue instruction; the bacc
legalization passes (move_matmul_waits_to_ldweights +
generate_event_semaphores) are run on the module to make the Tile-generated
sync legal.
"""

import numpy as np
import ml_dtypes

import concourse.bass as bass
import concourse.mybir as mybir
import concourse.tile as tile
from concourse.bass import ts
from concourse.bass_utils import run_bass_kernel_spmd

N_CORES = 8
T_FULL = 32768
N = 2048
T_CORE = T_FULL // N_CORES  # 4096
P = 128
ST = 512  # tokens per super-tile (one input DMA)

BF16 = mybir.dt.bfloat16
F32 = mybir.dt.float32


def _sylvester(n: int) -> np.ndarray:
    H = np.array([[1.0]], dtype=np.float64)
    while H.shape[0] < n:
        H = np.block([[H, H], [H, -H]])
    return H


def _host_consts() -> np.ndarray:
    """[128, 384] bf16: [R1p | H128s | I128].

    R1p = (I8 (x) H16)/16 with output columns permuted to j = mb*8 + ar so
    the m1 -> Y copy APs keep an 8-element contiguous innermost run.
    The total 1/sqrt(2048) output scale is folded into the constants:
    2^-4 into R1p (exact in bf16) and 2^-1.5 into H128s (~1e-4 rounding).
    """
    H16 = _sylvester(16)
    H128 = _sylvester(128)
    R1p = np.zeros((128, 128))
    for ar in range(8):
        for b in range(16):
            for mb in range(16):
                R1p[ar * 16 + b, mb * 8 + ar] = H16[b, mb] / 16.0
    ident = np.eye(128)
    return np.concatenate(
        [R1p, H128 * 2.0 ** -1.5, ident], axis=1
    ).astype(ml_dtypes.bfloat16)


def build_bass(t_core: int = T_CORE) -> bass.Bass:
    n_super = t_core // ST  # 8
    nc = bass.Bass()
    vt_p = nc.declare_dram_parameter("value_t", [N, t_core], BF16, isOutput=False)
    consts_p = nc.declare_dram_parameter("consts", [P, 3 * P], BF16, isOutput=False)
    out_p = nc.declare_dram_parameter("out", [t_core, N], BF16, isOutput=True)

    vt_view = vt_p.rearrange("(g p) t -> p g t", p=P)  # n = g*128 + p

    with tile.TileContext(nc) as tc:
        with (
            tc.tile_pool(name="consts", bufs=1) as consts,
            tc.tile_pool(name="vpool", bufs=4) as vpool,
            tc.tile_pool(name="ypool", bufs=5) as ypool,
            tc.tile_pool(name="v2pool", bufs=6) as v2pool,
            tc.tile_pool(name="opool", bufs=5) as opool,
            tc.tile_pool(name="m1psum", bufs=2, space="PSUM") as m1psum,
            tc.tile_pool(name="tpsum", bufs=2, space="PSUM") as tpsum,
            tc.tile_pool(name="m2psum", bufs=2, space="PSUM") as m2psum,
        ):
            CONSTS = consts.tile([P, 3 * P], BF16, tag="consts")
            # scalar (HWDGE) queue: runs parallel to the V loads on sync
            nc.scalar.dma_start(out=CONSTS, in_=consts_p[:, :])
            R1P = CONSTS[:, 0:P]
            H128B = CONSTS[:, P : 2 * P]
            IDENT = CONSTS[:, 2 * P : 3 * P]

            n_tiles = t_core // P

            V = None  # current super-tile [128, 16, 512]
            Ys = [None] * n_tiles  # per-tile stage-1 results [128, 16(mb), 128(a)]

            def emit_stage1(it):
                """MM1s for tile `it` -> Y[t, mb, a] in SBUF."""
                tsub = it % (ST // P)
                Y = ypool.tile([P, 16, P], BF16, tag="y")
                Yv = Y.rearrange("p m (g a) -> p g m a", g=16)  # [p,16,16,8]
                for G in range(2):
                    m1 = m1psum.tile([P, 8, P], F32, tag="m1")
                    for gg in range(8):
                        g = G * 8 + gg
                        nc.tensor.matmul(
                            m1[:, gg],
                            V[:, g, ts(tsub, P)],
                            R1P,
                            start=True,
                            stop=True,
                        )
                    # m1 free = (gg, mb, ar); Y slice free = (g, mb, ar).
                    # fp32 PSUM source -> 1x mode; strides are free at 1x.
                    # ACT is ~4x slower than DVE on strided copies, so all
                    # Y-assembly stays on DVE; FD=1024 amortizes the op
                    # overhead.
                    nc.vector.tensor_copy(
                        out=Yv[:, ts(G, 8)],
                        in_=m1.rearrange("p q (m a) -> p q m a", m=16),
                    )
                Ys[it] = Y

            def emit_stage2(it, split_store=False):
                """Transposes + MM2s for tile `it` -> store to DRAM.

                OUT free layout is (mb, ma) -- NOT final m = ma*16+mb order.
                Every PSUM drain is then contiguous; the host fixes the
                permutation during its bf16->f32 upcast pass.
                """
                Y = Ys[it]
                OUT = opool.tile([P, N], BF16, tag="o")
                for H in range(2):
                    tp2 = tpsum.tile([P, 8, P], BF16, tag="tp2")
                    for mm in range(8):
                        mb = H * 8 + mm
                        nc.tensor.transpose(tp2[:, mm], Y[:, mb, :], IDENT)
                    vt2 = v2pool.tile([P, 8, P], BF16, tag="vt2")
                    # ~1 in 4 tp2 drains go to ACT to balance DVE/ACT load
                    if H == 1 and it % 2 == 1:
                        nc.scalar.activation(
                            out=vt2,
                            in_=tp2,
                            func=mybir.ActivationFunctionType.Copy,
                        )
                    else:
                        nc.vector.tensor_copy(out=vt2, in_=tp2)
                    for q in range(2):
                        m2 = m2psum.tile([P, 4, P], F32, tag="m2")
                        for mm in range(4):
                            nc.tensor.matmul(
                                m2[:, mm],
                                vt2[:, q * 4 + mm],
                                H128B,
                                start=True,
                                stop=True,
                            )
                        Q = H * 2 + q
                        nc.scalar.activation(
                            out=OUT[:, ts(Q, 512)],
                            in_=m2,
                            func=mybir.ActivationFunctionType.Copy,
                        )
                        if split_store:
                            nc.gpsimd.dma_start(
                                out=out_p[ts(it, P), ts(Q, 512)],
                                in_=OUT[:, ts(Q, 512)],
                            )
                Ys[it] = None
                # stores ride the idle GpSimd (SWDGE) queue so their trigger
                # cost never delays the input prefetches on the Sync queue
                if not split_store:
                    nc.gpsimd.dma_start(out=out_p[ts(it, P), :], in_=OUT)

            # software pipeline: stage1(it) runs one tile ahead of stage2(it-1)
            # so PE never waits on the DVE copies that assemble Y. Input
            # super-tiles are prefetched one full super-tile (~13us of
            # compute) ahead of use.
            VS = [None] * n_super

            def issue_load(st):
                vtile = vpool.tile([P, 16, ST], BF16, tag="v")
                VS[st] = vtile
                if st == 0:
                    # split the first load into 256KB (g-half, t-quarter)
                    # chunks, in consumption order, so tile 0's first
                    # m1-group (g0-7, t0-127) starts after one chunk
                    for q in range(4):
                        for gh in range(2):
                            nc.sync.dma_start(
                                out=VS[st][:, ts(gh, 8), ts(q, P)],
                                in_=vt_view[:, ts(gh, 8), ts(q, P)],
                            )
                else:
                    nc.sync.dma_start(
                        out=VS[st], in_=vt_view[:, :, ts(st, ST)]
                    )

            issue_load(0)
            for st in range(n_super):
                V = VS[st]
                if st + 1 < n_super:
                    issue_load(st + 1)
                for tsub in range(ST // P):
                    it = st * (ST // P) + tsub
                    emit_stage1(it)
                    if it > 0:
                        emit_stage2(it - 1)
            emit_stage2(n_tiles - 1, split_store=True)

    # walrus allows at most 1 sem wait per PE-queue instruction; tile_legalize
    # pre-splits bf16 matmuls into Ldweights+Matmult, so run the bacc
    # legalization passes that spread/split the waits legally.
    import bass_rust

    bass_rust.move_matmul_waits_to_ldweights(nc.m)
    bass_rust.generate_event_semaphores(nc)
    return nc


_CACHE = {}


def _make_in_maps(inputs) -> list:
    value = np.asarray(inputs["value"])
    assert value.shape == (T_FULL, N), value.shape
    vb = value.astype(ml_dtypes.bfloat16)
    consts = _host_consts()
    in_maps = []
    for c in range(N_CORES):
        vt = np.ascontiguousarray(vb[c * T_CORE : (c + 1) * T_CORE].T)
        in_maps.append({"value_t": vt, "consts": consts})
    return in_maps


def _probe_ok(out: np.ndarray, inputs, n_rows: int = 3) -> bool:
    """Spot-check a few rows on the host against the *actual* weight input.

    Catches silent HW corruption and also the (unexpected) case of a
    non-Hadamard weight, falling back to the generic path.
    """
    value = np.asarray(inputs["value"])
    weight = np.asarray(inputs["weight"], dtype=np.float32)
    rows = np.linspace(0, T_FULL - 1, n_rows).astype(int)
    scale = np.float32(1.0 / np.sqrt(np.float32(weight.shape[0])))
    want = (np.asarray(value[rows], dtype=np.float32) @ weight) * scale
    got = out[rows]
    denom = max(float(np.abs(want).max()), 1e-30)
    rel = float(np.abs(got - want).max()) / denom
    return rel < 1.5e-2


def kernel(**inputs) -> np.ndarray:
    if "nc" not in _CACHE:
        _CACHE["nc"] = build_bass(T_CORE)
    nc = _CACHE["nc"]

    in_maps = _make_in_maps(inputs)
    try:
        out = None
        for attempt in range(2):
            res = run_bass_kernel_spmd(nc, in_maps, list(range(N_CORES)))
            o = np.concatenate([r["out"] for r in res.results], axis=0)
            # device emits (mb, ma)-ordered columns; final m = ma*16 + mb
            o = o.reshape(T_FULL, 16, 128).swapaxes(1, 2)
            o = np.ascontiguousarray(o, dtype=np.float32).reshape(T_FULL, N)
            if _probe_ok(o, inputs):
                out = o
                break
            print("kernel: probe mismatch on attempt", attempt, flush=True)
        if out is None:
            raise RuntimeError("bass kernel failed host probe twice")
        return out
    except Exception:
        import traceback

        traceback.print_exc()
        print("kernel: falling back to jax path", flush=True)
        # fallback: data-parallel GEMM via XLA on the 8 NeuronCores
        import jax
        import jax.numpy as jnp

        value = np.asarray(inputs["value"], dtype=np.float32)
        devs = jax.devices()[:N_CORES]
        scale = np.float32(1.0 / np.sqrt(np.float32(N)))
        w = np.asarray(inputs["weight"], dtype=np.float32)
        outs = []
        for c in range(N_CORES):
            d = devs[c % len(devs)]
            f = jax.jit(lambda a, b: jnp.dot(a, b) * scale, device=d)
            outs.append(f(value[c * T_CORE : (c + 1) * T_CORE], w))
        return np.concatenate([np.asarray(o) for o in outs], axis=0).astype(
            np.float32
        )
